# revision 9
# baseline (speedup 1.0000x reference)
"""Trainium2 Bass kernel for nn_DetectorKe_652835029279 (Gaussian-mixture
log-likelihood detector: weighted logsumexp over 256 Mahalanobis distances).

Math: ll_i = log sum_j coef_j * exp(-0.5 * (x_i-c_j)^T A_j (x_i-c_j)) - thr
    = logsumexp_j( -0.5 * x^T A_j x + x . (A_j c_j) + bias_j )
with bias_j = log(coef_j) - 0.5 c_j^T A_j c_j - thr folded in, and the
quadratic term expanded over the 17 cyclic-rotation pair blocks
(d, (d+k) % 32), k = 0..16 (544 pair slots; upper-triangle coverage with
doubled off-diagonal coefficients), so the whole row reduces to ONE matmul
  d'[i, j] = sum_s G[i, s] * U[s, j]
with G = [x_a * x_b (544 slots), x (32), 1] built on-chip, U precomputed on
host (tiny, M-sized).

Device layout per core (data-parallel over N, 16384 rows/core):
  per 512-row tile: DMA X -> PE-transpose to X^T [32,512] -> 6 selection
  matmuls build the rotated copies -> DVE multiplies build the pair products
  G^T chunks -> 6 accumulating f32r matmuls per 128-row group into PSUM
  [128,1024] -> ACT exp (+fused free-dim accumulate) -> Ln + PE transpose +
  single contiguous DMA out at the end.
"""
import sys

if "/opt/trn_rl_repo" not in sys.path:
    sys.path.insert(0, "/opt/trn_rl_repo")

import numpy as np

N, D, M = 131072, 32, 256
NCORES = 8
NC_ROWS = N // NCORES          # 16384
TILE_ROWS = 512
NTILES = NC_ROWS // TILE_ROWS  # 32
NGROUPS = NC_ROWS // 128       # 128

_PROGRAM = None


def _build_program():
    import concourse.bacc as bacc
    import concourse.mybir as mybir
    import concourse.tile as tile

    f32 = mybir.dt.float32
    f32r = mybir.dt.float32r
    AF = mybir.ActivationFunctionType

    nc = bacc.Bacc(None, target_bir_lowering=False)
    X_d = nc.dram_tensor("X", [NC_ROWS, D], f32r, kind="ExternalInput")
    U_d = nc.dram_tensor("U", [128, 5, M], f32r, kind="ExternalInput")
    Uxb_d = nc.dram_tensor("Uxb", [33, M], f32r, kind="ExternalInput")
    SEL_d = nc.dram_tensor("SEL", [32, 672], f32r, kind="ExternalInput")
    ONES_d = nc.dram_tensor("ONES", [1, TILE_ROWS], f32r, kind="ExternalInput")
    EYE_d = nc.dram_tensor("EYE", [128, 128], f32, kind="ExternalInput")
    EYER_d = nc.dram_tensor("EYER", [128, 128], f32r, kind="ExternalInput")
    OUT_d = nc.dram_tensor("out", [NC_ROWS], f32, kind="ExternalOutput")

    with tile.TileContext(nc) as tc:
        with (
            tc.tile_pool(name="const", bufs=1) as constp,
            tc.tile_pool(name="xin", bufs=3) as xinp,
            tc.tile_pool(name="xtp", bufs=2) as xtpool,
            tc.tile_pool(name="xt4p", bufs=2) as xt4pool,
            tc.tile_pool(name="xxp", bufs=2) as xxpool,
            tc.tile_pool(name="expp", bufs=4) as exppool,
            tc.tile_pool(name="sumsp", bufs=1) as sumspool,
            tc.tile_pool(name="finp", bufs=1) as finpool,
            tc.tile_pool(name="ps_xt", bufs=1, space="PSUM") as ps_xt,
            tc.tile_pool(name="ps_xt4", bufs=1, space="PSUM") as ps_xt4,
            tc.tile_pool(name="ps_rot", bufs=2, space="PSUM") as ps_rot,
            tc.tile_pool(name="ps_main", bufs=2, space="PSUM") as ps_main,
        ):
            U_sb = constp.tile([128, 5, M], f32r)
            nc.sync.dma_start(U_sb[:], U_d[:])
            Uxb_sb = constp.tile([33, M], f32r)
            nc.sync.dma_start(Uxb_sb[:], Uxb_d[:])
            SEL_sb = constp.tile([32, 672], f32r)
            nc.sync.dma_start(SEL_sb[:], SEL_d[:])
            EYE_sb = constp.tile([128, 128], f32)
            nc.sync.dma_start(EYE_sb[:], EYE_d[:])
            EYER_sb = constp.tile([128, 128], f32r)
            nc.sync.dma_start(EYER_sb[:], EYER_d[:])

            sums_sb = sumspool.tile([128, NGROUPS], f32)

            for t in range(NTILES):
                x_t = xinp.tile([128, 4 * D], f32r, tag="x")
                nc.sync.dma_start(
                    x_t[:].rearrange("p (g d) -> p g d", g=4),
                    X_d[t * TILE_ROWS : (t + 1) * TILE_ROWS, :].rearrange(
                        "(g p) d -> p g d", p=128
                    ),
                )

                # X^T [32, 512] via 4 PE transposes
                xtps = ps_xt.tile([32, TILE_ROWS], f32r, tag="xtps")
                for g in range(4):
                    nc.tensor.transpose(
                        xtps[:, g * 128 : (g + 1) * 128],
                        x_t[:, g * D : (g + 1) * D],
                        EYER_sb[:],
                    )
                # rows 0:32 = X^T; row 32 = ones (for the bias matmul)
                xt_sb = xtpool.tile([33, TILE_ROWS], f32r, tag="xt")
                nc.scalar.copy(xt_sb[:32, :], xtps[:])
                nc.sync.dma_start(xt_sb[32:33, :], ONES_d[:])

                # XT4 = 4-fold stack of X^T (partition p holds x_{p%32})
                xt4ps = ps_xt4.tile([128, TILE_ROWS], f32, tag="xt4ps")
                nc.tensor.matmul(
                    xt4ps[:],
                    SEL_sb[:, 0:128],
                    xt_sb[:32, :],
                    start=True,
                    stop=True,
                )
                xt4_sb = xt4pool.tile([128, TILE_ROWS], f32r, tag="xt4")
                nc.scalar.copy(xt4_sb[:], xt4ps[:])

                # pair-product chunks: xx_c[p] = x_{p%32} * x_{(p%32 + 4c + p//32)%32}
                xx_tiles = []
                for c in range(4):
                    rotps = ps_rot.tile([128, TILE_ROWS], f32, tag="rot")
                    nc.tensor.matmul(
                        rotps[:],
                        SEL_sb[:, 128 * (c + 1) : 128 * (c + 2)],
                        xt_sb[:32, :],
                        start=True,
                        stop=True,
                    )
                    xx_c = xxpool.tile([128, TILE_ROWS], f32r, tag=f"xx{c}")
                    nc.vector.tensor_mul(xx_c[:], xt4_sb[:], rotps[:])
                    xx_tiles.append(xx_c)

                rot16ps = ps_rot.tile([32, TILE_ROWS], f32, tag="rot")
                nc.tensor.matmul(
                    rot16ps[:],
                    SEL_sb[:, 640:672],
                    xt_sb[:32, :],
                    start=True,
                    stop=True,
                )
                xx4 = xxpool.tile([32, TILE_ROWS], f32r, tag="xx4")
                nc.vector.tensor_mul(xx4[:], xt_sb[:32, :], rot16ps[:])

                # main matmuls: 4 groups of 128 rows, 256 clusters each
                psmain = ps_main.tile([128, 4 * M], f32, tag="main")
                for sub in range(4):
                    sl = psmain[:, sub * M : (sub + 1) * M]
                    for c in range(4):
                        nc.tensor.matmul(
                            sl,
                            xx_tiles[c][:, sub * 128 : (sub + 1) * 128],
                            U_sb[:, c, :],
                            start=(c == 0),
                            stop=False,
                        )
                    nc.tensor.matmul(
                        sl,
                        xx4[:, sub * 128 : (sub + 1) * 128],
                        U_sb[:32, 4, :],
                        start=False,
                        stop=False,
                    )
                    nc.tensor.matmul(
                        sl,
                        xt_sb[:, sub * 128 : (sub + 1) * 128],
                        Uxb_sb[:],
                        start=False,
                        stop=True,
                    )

                # exp + fused row-sum into the per-core sums accumulator
                for sub in range(4):
                    expsc = exppool.tile([128, M], f32, tag="exp")
                    col = t * 4 + sub
                    nc.scalar.activation(
                        expsc[:],
                        psmain[:, sub * M : (sub + 1) * M],
                        AF.Exp,
                        accum_out=sums_sb[:, col : col + 1],
                    )

            # epilogue: ll^T = Ln(sums); transpose; contiguous DMA out
            llT = finpool.tile([128, NGROUPS], f32)
            nc.scalar.activation(llT[:], sums_sb[:], AF.Ln)
            llps = ps_xt.tile([128, 128], f32, tag="xtps")
            nc.tensor.transpose(llps[:], llT[:], EYE_sb[:])
            ll_sb = finpool.tile([128, 128], f32)
            nc.scalar.copy(ll_sb[:], llps[:])
            nc.sync.dma_start(OUT_d.rearrange("(c p) -> c p", c=128), ll_sb[:])

    nc.compile()
    return nc


def _host_prep(center, cov_inv_sqrt, weight, threshold):
    L = np.asarray(cov_inv_sqrt, dtype=np.float64)
    w = np.abs(np.asarray(weight, dtype=np.float64))
    pr = w / w.sum()
    A = np.einsum("mij,mkj->mik", L, L)
    sign, logdet = np.linalg.slogdet(A)
    logcoef = np.log(pr) + 0.5 * logdet
    c64 = np.asarray(center, dtype=np.float64)
    Ac = np.einsum("mkl,ml->mk", A, c64)
    term3 = np.einsum("mk,mk->m", c64, Ac)
    bias = logcoef - 0.5 * term3 - float(np.asarray(threshold).reshape(-1)[0])

    U = np.zeros((128, 5, M), np.float32)
    p = np.arange(128)
    for c in range(4):
        k = 4 * c + p // 32
        d1 = p % 32
        d2 = (d1 + k) % 32
        mult = np.where((k == 0) | (k == 16), 1.0, 2.0)
        U[:, c, :] = (-0.5 * mult[:, None] * A[:, d1, d2].T).astype(np.float32)
    p32 = np.arange(32)
    U[:32, 4, :] = (-0.5 * A[:, p32, (p32 + 16) % 32].T).astype(np.float32)

    Uxb = np.zeros((33, M), np.float32)
    Uxb[:32, :] = Ac.T.astype(np.float32)
    Uxb[32, :] = bias.astype(np.float32)

    SEL = np.zeros((32, 672), np.float32)
    dd = np.arange(32)
    SEL[:, 0:128] = (dd[:, None] == (p[None, :] % 32)).astype(np.float32)
    for c in range(4):
        k = 4 * c + p // 32
        b = (p % 32 + k) % 32
        SEL[:, 128 * (c + 1) : 128 * (c + 2)] = (dd[:, None] == b[None, :]).astype(
            np.float32
        )
    SEL[:, 640:672] = (dd[:, None] == ((p32[None, :] + 16) % 32)).astype(np.float32)

    EYE = np.eye(128, dtype=np.float32)
    return U, Uxb, SEL, EYE


def kernel(X, center, cov_inv_sqrt, weight, threshold):
    global _PROGRAM
    from concourse.bass_utils import run_bass_kernel_spmd

    X = np.ascontiguousarray(np.asarray(X, dtype=np.float32))
    U, Uxb, SEL, EYE = _host_prep(center, cov_inv_sqrt, weight, threshold)

    if _PROGRAM is None:
        _PROGRAM = _build_program()
    nc = _PROGRAM

    in_maps = []
    for k in range(NCORES):
        in_maps.append(
            {
                "X": X[k * NC_ROWS : (k + 1) * NC_ROWS],
                "U": U,
                "Uxb": Uxb,
                "SEL": SEL,
                "ONES": np.ones((1, TILE_ROWS), np.float32),
                "EYE": EYE,
                "EYER": EYE,
            }
        )
    res = run_bass_kernel_spmd(nc, in_maps, list(range(NCORES)))
    out = np.concatenate([res.results[k]["out"] for k in range(NCORES)])
    return out.astype(np.float32)


# revision 11
# speedup vs baseline: 1.7399x; 1.7399x over previous
"""Trainium2 Bass kernel for nn_DetectorKe_652835029279 (Gaussian-mixture
log-likelihood detector: weighted logsumexp over 256 Mahalanobis distances).

Math: ll_i = log sum_j coef_j * exp(-0.5 * (x_i-c_j)^T A_j (x_i-c_j)) - thr
    = logsumexp_j( -0.5 * x^T A_j x + x . (A_j c_j) + bias_j )
with bias_j = log(coef_j) - 0.5 c_j^T A_j c_j - thr folded in, and the
quadratic term expanded over the 17 cyclic-rotation pair blocks
(d, (d+k) % 32), k = 0..16 (544 pair slots; upper-triangle coverage with
doubled off-diagonal coefficients), so the whole row reduces to ONE matmul
  d'[i, j] = sum_s G[i, s] * U[s, j]
with G = [x_a * x_b (544 slots), x (32), 1, zero-pad] built on-chip and U
precomputed on host (tiny, M-sized).

All matmuls are float32r (fp22 read-truncation, ~1 cycle/row) and K-padded
to 128 partitions (K<128 runs at half rate on trn2) - pad rows are exact
zeros on both operands so they contribute nothing.

Device layout per core (data-parallel over N, 16384 rows/core), per
512-row tile: DMA X -> 4 PE transposes to X^T [32,512] -> 6 padded
selection matmuls build rotated copies -> 5 DVE multiplies build the pair
products -> 24 accumulating K=128 matmuls (chunk-outer order, one PSUM
wait per chunk) into PSUM [128,1024] -> ACT exp with fused free-dim
accumulate -> Ln + PE transpose + contiguous DMA out at the end.
"""
import sys

if "/opt/trn_rl_repo" not in sys.path:
    sys.path.insert(0, "/opt/trn_rl_repo")

import numpy as np

N, D, M = 131072, 32, 256
NCORES = 8
NC_ROWS = N // NCORES          # 16384
TILE_ROWS = 512
NTILES = NC_ROWS // TILE_ROWS  # 32
NGROUPS = NC_ROWS // 128       # 128
NCHUNK = 6

_PROGRAM = None


def _build_program():
    import concourse.bacc as bacc
    import concourse.mybir as mybir
    import concourse.tile as tile

    f32 = mybir.dt.float32
    f32r = mybir.dt.float32r
    AF = mybir.ActivationFunctionType

    nc = bacc.Bacc(None, target_bir_lowering=False)
    X_d = nc.dram_tensor("X", [NC_ROWS, D], f32r, kind="ExternalInput")
    U_d = nc.dram_tensor("U", [128, NCHUNK, M], f32r, kind="ExternalInput")
    SEL_d = nc.dram_tensor("SEL", [128, 768], f32r, kind="ExternalInput")
    PAD_d = nc.dram_tensor("PAD", [96, TILE_ROWS], f32r, kind="ExternalInput")
    EYE_d = nc.dram_tensor("EYE", [128, 128], f32, kind="ExternalInput")
    EYER_d = nc.dram_tensor("EYER", [128, 128], f32r, kind="ExternalInput")
    OUT_d = nc.dram_tensor("out", [NC_ROWS], f32, kind="ExternalOutput")

    with tile.TileContext(nc) as tc:
        with (
            tc.tile_pool(name="const", bufs=1) as constp,
            tc.tile_pool(name="xin", bufs=3) as xinp,
            tc.tile_pool(name="xtp", bufs=2) as xtpool,
            tc.tile_pool(name="xt4p", bufs=2) as xt4pool,
            tc.tile_pool(name="xxp", bufs=2) as xxpool,
            tc.tile_pool(name="expp", bufs=4) as exppool,
            tc.tile_pool(name="sumsp", bufs=1) as sumspool,
            tc.tile_pool(name="finp", bufs=1) as finpool,
            tc.tile_pool(name="ps_xt", bufs=1, space="PSUM") as ps_xt,
            tc.tile_pool(name="ps_xt4", bufs=1, space="PSUM") as ps_xt4,
            tc.tile_pool(name="ps_rot", bufs=2, space="PSUM") as ps_rot,
            tc.tile_pool(name="ps_main", bufs=2, space="PSUM") as ps_main,
        ):
            U_sb = constp.tile([128, NCHUNK, M], f32r)
            nc.sync.dma_start(U_sb[:], U_d[:])
            SEL_sb = constp.tile([128, 768], f32r)
            nc.sync.dma_start(SEL_sb[:], SEL_d[:])
            EYE_sb = constp.tile([128, 128], f32)
            nc.sync.dma_start(EYE_sb[:], EYE_d[:])
            EYER_sb = constp.tile([128, 128], f32r)
            nc.sync.dma_start(EYER_sb[:], EYER_d[:])

            sums_sb = sumspool.tile([128, NGROUPS], f32)

            for t in range(NTILES):
                x_t = xinp.tile([128, 4 * D], f32r, tag="x")
                nc.sync.dma_start(
                    x_t[:].rearrange("p (g d) -> p g d", g=4),
                    X_d[t * TILE_ROWS : (t + 1) * TILE_ROWS, :].rearrange(
                        "(g p) d -> p g d", p=128
                    ),
                )

                # X^T [32, 512] via 4 PE transposes
                xtps = ps_xt.tile([32, TILE_ROWS], f32r, tag="xtps")
                for g in range(4):
                    nc.tensor.transpose(
                        xtps[:, g * 128 : (g + 1) * 128],
                        x_t[:, g * D : (g + 1) * D],
                        EYER_sb[:],
                    )
                # xt_sb = [X^T (32) ; ones (1) ; zeros (95)] - serves both as
                # the sel-matmul moving operand (rows 32:128 exactly zero) and
                # as main-matmul chunk 5 (x-linear part + bias row).
                xt_sb = xtpool.tile([128, TILE_ROWS], f32r, tag="xt")
                nc.scalar.copy(xt_sb[:32, :], xtps[:])
                nc.sync.dma_start(xt_sb[32:128, :], PAD_d[:])

                # XT4 = 4-fold stack of X^T (partition p holds x_{p%32})
                xt4ps = ps_xt4.tile([128, TILE_ROWS], f32, tag="xt4ps")
                nc.tensor.matmul(
                    xt4ps[:], SEL_sb[:, 0:128], xt_sb[:], start=True, stop=True
                )
                xt4_sb = xt4pool.tile([128, TILE_ROWS], f32r, tag="xt4")
                nc.scalar.copy(xt4_sb[:], xt4ps[:])

                # pair-product chunks 0..3:
                #   chunk_c[p] = x_{p%32} * x_{(p%32 + 4c + p//32)%32}
                # chunk 4: k=16 block in rows 0:32, rows 32:128 exact zeros
                # (sel rows are zero there, and xt4 * 0 = 0).
                chunk_tiles = []
                for c in range(5):
                    rotps = ps_rot.tile([128, TILE_ROWS], f32, tag="rot")
                    nc.tensor.matmul(
                        rotps[:],
                        SEL_sb[:, 128 * (c + 1) : 128 * (c + 2)],
                        xt_sb[:],
                        start=True,
                        stop=True,
                    )
                    xx_c = xxpool.tile([128, TILE_ROWS], f32r, tag=f"xx{c}")
                    nc.vector.tensor_mul(xx_c[:], xt4_sb[:], rotps[:])
                    chunk_tiles.append(xx_c)
                chunk_tiles.append(xt_sb)  # chunk 5: [X^T; ones; zeros]

                # main accumulating matmuls (one open PSUM group per bank)
                psmain = ps_main.tile([128, 4 * M], f32, tag="main")
                for sub in range(4):
                    for c in range(NCHUNK):
                        nc.tensor.matmul(
                            psmain[:, sub * M : (sub + 1) * M],
                            chunk_tiles[c][:, sub * 128 : (sub + 1) * 128],
                            U_sb[:, c, :],
                            start=(c == 0),
                            stop=(c == NCHUNK - 1),
                        )

                # exp + fused row-sum into the per-core sums accumulator
                for sub in range(4):
                    expsc = exppool.tile([128, M], f32, tag="exp")
                    col = t * 4 + sub
                    nc.scalar.activation(
                        expsc[:],
                        psmain[:, sub * M : (sub + 1) * M],
                        AF.Exp,
                        accum_out=sums_sb[:, col : col + 1],
                    )

            # epilogue: ll^T = Ln(sums); transpose; contiguous DMA out
            llT = finpool.tile([128, NGROUPS], f32)
            nc.scalar.activation(llT[:], sums_sb[:], AF.Ln)
            llps = ps_xt.tile([128, 128], f32, tag="xtps")
            nc.tensor.transpose(llps[:], llT[:], EYE_sb[:])
            ll_sb = finpool.tile([128, 128], f32)
            nc.scalar.copy(ll_sb[:], llps[:])
            nc.sync.dma_start(OUT_d.rearrange("(c p) -> c p", c=128), ll_sb[:])

    nc.compile()
    return nc


def _host_prep(center, cov_inv_sqrt, weight, threshold):
    L = np.asarray(cov_inv_sqrt, dtype=np.float64)
    w = np.abs(np.asarray(weight, dtype=np.float64))
    pr = w / w.sum()
    A = np.einsum("mij,mkj->mik", L, L)
    sign, logdet = np.linalg.slogdet(A)
    logcoef = np.log(pr) + 0.5 * logdet
    c64 = np.asarray(center, dtype=np.float64)
    Ac = np.einsum("mkl,ml->mk", A, c64)
    term3 = np.einsum("mk,mk->m", c64, Ac)
    bias = logcoef - 0.5 * term3 - float(np.asarray(threshold).reshape(-1)[0])

    U = np.zeros((128, NCHUNK, M), np.float32)
    p = np.arange(128)
    for c in range(4):
        k = 4 * c + p // 32
        d1 = p % 32
        d2 = (d1 + k) % 32
        mult = np.where((k == 0) | (k == 16), 1.0, 2.0)
        U[:, c, :] = (-0.5 * mult[:, None] * A[:, d1, d2].T).astype(np.float32)
    p32 = np.arange(32)
    U[:32, 4, :] = (-0.5 * A[:, p32, (p32 + 16) % 32].T).astype(np.float32)
    U[:32, 5, :] = Ac.T.astype(np.float32)
    U[32, 5, :] = bias.astype(np.float32)

    SEL = np.zeros((128, 768), np.float32)
    dd = np.arange(128)
    SEL[:, 0:128] = (dd[:, None] == (p[None, :] % 32)).astype(np.float32)
    for c in range(4):
        k = 4 * c + p // 32
        b = (p % 32 + k) % 32
        SEL[:, 128 * (c + 1) : 128 * (c + 2)] = (dd[:, None] == b[None, :]).astype(
            np.float32
        )
    b16 = np.where(p < 32, (p + 16) % 32, -1)
    SEL[:, 640:768] = (dd[:, None] == b16[None, :]).astype(np.float32)

    PAD = np.zeros((96, TILE_ROWS), np.float32)
    PAD[0, :] = 1.0
    EYE = np.eye(128, dtype=np.float32)
    return U, SEL, PAD, EYE


def kernel(X, center, cov_inv_sqrt, weight, threshold):
    global _PROGRAM
    from concourse.bass_utils import run_bass_kernel_spmd

    X = np.ascontiguousarray(np.asarray(X, dtype=np.float32))
    U, SEL, PAD, EYE = _host_prep(center, cov_inv_sqrt, weight, threshold)

    if _PROGRAM is None:
        _PROGRAM = _build_program()
    nc = _PROGRAM

    in_maps = []
    for k in range(NCORES):
        in_maps.append(
            {
                "X": X[k * NC_ROWS : (k + 1) * NC_ROWS],
                "U": U,
                "SEL": SEL,
                "PAD": PAD,
                "EYE": EYE,
                "EYER": EYE,
            }
        )
    res = run_bass_kernel_spmd(nc, in_maps, list(range(NCORES)))
    out = np.concatenate([res.results[k]["out"] for k in range(NCORES)])
    return out.astype(np.float32)


# revision 13
# speedup vs baseline: 2.1464x; 1.2336x over previous
"""Trainium2 Bass kernel for nn_DetectorKe_652835029279 (Gaussian-mixture
log-likelihood detector: weighted logsumexp over 256 Mahalanobis distances).

Math: ll_i = log sum_j coef_j * exp(-0.5 * (x_i-c_j)^T A_j (x_i-c_j)) - thr
    = logsumexp_j( -0.5 * x^T A_j x + x . (A_j c_j) + bias_j )
with bias_j = log(coef_j) - 0.5 c_j^T A_j c_j - thr folded in, and the
quadratic term expanded over the 17 cyclic-rotation pair blocks
(d, (d+k) % 32), k = 0..16 (544 pair slots; upper-triangle coverage with
doubled off-diagonal coefficients), so the whole row reduces to ONE matmul
  d'[i, j] = sum_s G[i, s] * U[s, j]
with G = [x_a * x_b (544 slots), x (32), 1, zero-pad] built on-chip and U
precomputed on host (tiny, M-sized).

All matmuls are float32r (fp22 read-truncation, ~1 cycle/row) and K-padded
to 128 partitions (K<128 runs at half rate on trn2) - pad rows are exact
zeros on both operands so they contribute nothing.

Device layout per core (data-parallel over N, 16384 rows/core), per
512-row tile: DMA X -> 4 PE transposes to X^T [32,512] -> 6 padded
selection matmuls build rotated copies -> 5 DVE multiplies build the pair
products -> 24 accumulating K=128 matmuls (chunk-outer order, one PSUM
wait per chunk) into PSUM [128,1024] -> ACT exp with fused free-dim
accumulate -> Ln + PE transpose + contiguous DMA out at the end.
"""
import sys

if "/opt/trn_rl_repo" not in sys.path:
    sys.path.insert(0, "/opt/trn_rl_repo")

import numpy as np

N, D, M = 131072, 32, 256
NCORES = 8
NC_ROWS = N // NCORES          # 16384
TILE_ROWS = 512
NTILES = NC_ROWS // TILE_ROWS  # 32
NGROUPS = NC_ROWS // 128       # 128
NCHUNK = 6

_PROGRAM = None


def _build_program():
    import concourse.bacc as bacc
    import concourse.mybir as mybir
    import concourse.tile as tile

    f32 = mybir.dt.float32
    f32r = mybir.dt.float32r
    AF = mybir.ActivationFunctionType

    nc = bacc.Bacc(None, target_bir_lowering=False)
    X_d = nc.dram_tensor("X", [NC_ROWS, D], f32r, kind="ExternalInput")
    U_d = nc.dram_tensor("U", [128, NCHUNK, M], f32r, kind="ExternalInput")
    SEL_d = nc.dram_tensor("SEL", [128, 768], f32r, kind="ExternalInput")
    PAD_d = nc.dram_tensor("PAD", [96, TILE_ROWS], f32r, kind="ExternalInput")
    EYE_d = nc.dram_tensor("EYE", [128, 128], f32, kind="ExternalInput")
    EYER_d = nc.dram_tensor("EYER", [128, 128], f32r, kind="ExternalInput")
    OUT_d = nc.dram_tensor("out", [NC_ROWS], f32, kind="ExternalOutput")

    with tile.TileContext(nc) as tc:
        with (
            tc.tile_pool(name="const", bufs=1) as constp,
            tc.tile_pool(name="xin", bufs=3) as xinp,
            tc.tile_pool(name="xtp", bufs=2) as xtpool,
            tc.tile_pool(name="xt4p", bufs=2) as xt4pool,
            tc.tile_pool(name="xxp", bufs=2) as xxpool,
            tc.tile_pool(name="expp", bufs=4) as exppool,
            tc.tile_pool(name="sumsp", bufs=1) as sumspool,
            tc.tile_pool(name="finp", bufs=1) as finpool,
            tc.tile_pool(name="ps_xt", bufs=1, space="PSUM") as ps_xt,
            tc.tile_pool(name="ps_xt4", bufs=1, space="PSUM") as ps_xt4,
            tc.tile_pool(name="ps_rot", bufs=2, space="PSUM") as ps_rot,
            tc.tile_pool(name="ps_main", bufs=2, space="PSUM") as ps_main,
        ):
            U_sb = constp.tile([128, NCHUNK, M], f32r)
            nc.sync.dma_start(U_sb[:], U_d[:])
            SEL_sb = constp.tile([128, 768], f32r)
            nc.sync.dma_start(SEL_sb[:], SEL_d[:])
            EYE_sb = constp.tile([128, 128], f32)
            nc.sync.dma_start(EYE_sb[:], EYE_d[:])
            EYER_sb = constp.tile([128, 128], f32r)
            nc.sync.dma_start(EYER_sb[:], EYER_d[:])

            sums_sb = sumspool.tile([128, NGROUPS], f32)

            # persistent double-buffered X^T tiles: rows 32:128 hold the
            # constant [ones-row; zeros] pad, DMA'd once - per-tile writes
            # only touch rows 0:32, so the pad stays valid across reuse.
            xt_tiles = []
            for i in range(2):
                xt_p = xtpool.tile(
                    [128, TILE_ROWS], f32r, tag=f"xtP{i}", bufs=1, name=f"xt_p{i}"
                )
                nc.sync.dma_start(xt_p[32:128, :], PAD_d[:])
                xt_tiles.append(xt_p)

            for t in range(NTILES):
                x_t = xinp.tile([128, 4 * D], f32r, tag="x")
                nc.sync.dma_start(
                    x_t[:].rearrange("p (g d) -> p g d", g=4),
                    X_d[t * TILE_ROWS : (t + 1) * TILE_ROWS, :].rearrange(
                        "(g p) d -> p g d", p=128
                    ),
                )

                # X^T [32, 512] via 4 PE transposes
                xtps = ps_xt.tile([32, TILE_ROWS], f32r, tag="xtps")
                for g in range(4):
                    nc.tensor.transpose(
                        xtps[:, g * 128 : (g + 1) * 128],
                        x_t[:, g * D : (g + 1) * D],
                        EYER_sb[:],
                    )
                # xt_sb = [X^T (32) ; ones (1) ; zeros (95)] - serves both as
                # the sel-matmul moving operand (rows 32:128 exactly zero) and
                # as main-matmul chunk 5 (x-linear part + bias row).
                xt_sb = xt_tiles[t % 2]
                nc.scalar.copy(xt_sb[:32, :], xtps[:])

                # XT4 = 4-fold stack of X^T (partition p holds x_{p%32})
                xt4ps = ps_xt4.tile([128, TILE_ROWS], f32, tag="xt4ps")
                nc.tensor.matmul(
                    xt4ps[:], SEL_sb[:, 0:128], xt_sb[:], start=True, stop=True
                )
                xt4_sb = xt4pool.tile([128, TILE_ROWS], f32r, tag="xt4")
                nc.scalar.copy(xt4_sb[:], xt4ps[:])

                # pair-product chunks 0..3:
                #   chunk_c[p] = x_{p%32} * x_{(p%32 + 4c + p//32)%32}
                # chunk 4: k=16 block in rows 0:32, rows 32:128 exact zeros
                # (sel rows are zero there, and xt4 * 0 = 0).
                chunk_tiles = []
                for c in range(5):
                    rotps = ps_rot.tile([128, TILE_ROWS], f32, tag="rot")
                    nc.tensor.matmul(
                        rotps[:],
                        SEL_sb[:, 128 * (c + 1) : 128 * (c + 2)],
                        xt_sb[:],
                        start=True,
                        stop=True,
                    )
                    xx_c = xxpool.tile([128, TILE_ROWS], f32r, tag=f"xx{c}")
                    nc.vector.tensor_mul(xx_c[:], xt4_sb[:], rotps[:])
                    chunk_tiles.append(xx_c)
                chunk_tiles.append(xt_sb)  # chunk 5: [X^T; ones; zeros]

                # main accumulating matmuls (one open PSUM group per bank)
                psmain = ps_main.tile([128, 4 * M], f32, tag="main")
                for sub in range(4):
                    for c in range(NCHUNK):
                        nc.tensor.matmul(
                            psmain[:, sub * M : (sub + 1) * M],
                            chunk_tiles[c][:, sub * 128 : (sub + 1) * 128],
                            U_sb[:, c, :],
                            start=(c == 0),
                            stop=(c == NCHUNK - 1),
                        )

                # exp + fused row-sum into the per-core sums accumulator
                for sub in range(4):
                    expsc = exppool.tile([128, M], f32, tag="exp")
                    col = t * 4 + sub
                    nc.scalar.activation(
                        expsc[:],
                        psmain[:, sub * M : (sub + 1) * M],
                        AF.Exp,
                        accum_out=sums_sb[:, col : col + 1],
                    )

            # epilogue: ll^T = Ln(sums); transpose; contiguous DMA out
            llT = finpool.tile([128, NGROUPS], f32)
            nc.scalar.activation(llT[:], sums_sb[:], AF.Ln)
            llps = ps_xt.tile([128, 128], f32, tag="xtps")
            nc.tensor.transpose(llps[:], llT[:], EYE_sb[:])
            ll_sb = finpool.tile([128, 128], f32)
            nc.scalar.copy(ll_sb[:], llps[:])
            nc.sync.dma_start(OUT_d.rearrange("(c p) -> c p", c=128), ll_sb[:])

    nc.compile()
    return nc


def _host_prep(center, cov_inv_sqrt, weight, threshold):
    L = np.asarray(cov_inv_sqrt, dtype=np.float64)
    w = np.abs(np.asarray(weight, dtype=np.float64))
    pr = w / w.sum()
    A = np.einsum("mij,mkj->mik", L, L)
    sign, logdet = np.linalg.slogdet(A)
    logcoef = np.log(pr) + 0.5 * logdet
    c64 = np.asarray(center, dtype=np.float64)
    Ac = np.einsum("mkl,ml->mk", A, c64)
    term3 = np.einsum("mk,mk->m", c64, Ac)
    bias = logcoef - 0.5 * term3 - float(np.asarray(threshold).reshape(-1)[0])

    U = np.zeros((128, NCHUNK, M), np.float32)
    p = np.arange(128)
    for c in range(4):
        k = 4 * c + p // 32
        d1 = p % 32
        d2 = (d1 + k) % 32
        mult = np.where((k == 0) | (k == 16), 1.0, 2.0)
        U[:, c, :] = (-0.5 * mult[:, None] * A[:, d1, d2].T).astype(np.float32)
    p32 = np.arange(32)
    U[:32, 4, :] = (-0.5 * A[:, p32, (p32 + 16) % 32].T).astype(np.float32)
    U[:32, 5, :] = Ac.T.astype(np.float32)
    U[32, 5, :] = bias.astype(np.float32)

    SEL = np.zeros((128, 768), np.float32)
    dd = np.arange(128)
    SEL[:, 0:128] = (dd[:, None] == (p[None, :] % 32)).astype(np.float32)
    for c in range(4):
        k = 4 * c + p // 32
        b = (p % 32 + k) % 32
        SEL[:, 128 * (c + 1) : 128 * (c + 2)] = (dd[:, None] == b[None, :]).astype(
            np.float32
        )
    b16 = np.where(p < 32, (p + 16) % 32, -1)
    SEL[:, 640:768] = (dd[:, None] == b16[None, :]).astype(np.float32)

    PAD = np.zeros((96, TILE_ROWS), np.float32)
    PAD[0, :] = 1.0
    EYE = np.eye(128, dtype=np.float32)
    return U, SEL, PAD, EYE


def kernel(X, center, cov_inv_sqrt, weight, threshold):
    global _PROGRAM
    from concourse.bass_utils import run_bass_kernel_spmd

    X = np.ascontiguousarray(np.asarray(X, dtype=np.float32))
    U, SEL, PAD, EYE = _host_prep(center, cov_inv_sqrt, weight, threshold)

    if _PROGRAM is None:
        _PROGRAM = _build_program()
    nc = _PROGRAM

    in_maps = []
    for k in range(NCORES):
        in_maps.append(
            {
                "X": X[k * NC_ROWS : (k + 1) * NC_ROWS],
                "U": U,
                "SEL": SEL,
                "PAD": PAD,
                "EYE": EYE,
                "EYER": EYE,
            }
        )
    res = run_bass_kernel_spmd(nc, in_maps, list(range(NCORES)))
    out = np.concatenate([res.results[k]["out"] for k in range(NCORES)])
    return out.astype(np.float32)


# revision 14
# speedup vs baseline: 2.3098x; 1.0762x over previous
"""Trainium2 Bass kernel for nn_DetectorKe_652835029279 (Gaussian-mixture
log-likelihood detector: weighted logsumexp over 256 Mahalanobis distances).

Math: ll_i = log sum_j coef_j * exp(-0.5 * (x_i-c_j)^T A_j (x_i-c_j)) - thr
    = logsumexp_j( -0.5 * x^T A_j x + x . (A_j c_j) + bias_j )
with bias_j = log(coef_j) - 0.5 c_j^T A_j c_j - thr folded in, and the
quadratic term expanded over the 17 cyclic-rotation pair blocks
(d, (d+k) % 32), k = 0..16 (544 pair slots; upper-triangle coverage with
doubled off-diagonal coefficients), so the whole row reduces to ONE matmul
  d'[i, j] = sum_s G[i, s] * U[s, j]
with G = [x_a * x_b (544 slots), x (32), 1, zero-pad] built on-chip and U
precomputed on host (tiny, M-sized).

All matmuls are float32r (fp22 read-truncation, ~1 cycle/row) and K-padded
to 128 partitions (K<128 runs at half rate on trn2) - pad rows are exact
zeros on both operands so they contribute nothing.

Device layout per core (data-parallel over N, 16384 rows/core), per
512-row tile: DMA X -> 4 PE transposes to X^T [32,512] -> 6 padded
selection matmuls build rotated copies -> 5 DVE multiplies build the pair
products -> 24 accumulating K=128 matmuls (chunk-outer order, one PSUM
wait per chunk) into PSUM [128,1024] -> ACT exp with fused free-dim
accumulate -> Ln + PE transpose + contiguous DMA out at the end.
"""
import sys

if "/opt/trn_rl_repo" not in sys.path:
    sys.path.insert(0, "/opt/trn_rl_repo")

import numpy as np

N, D, M = 131072, 32, 256
NCORES = 8
NC_ROWS = N // NCORES          # 16384
TILE_ROWS = 512
NTILES = NC_ROWS // TILE_ROWS  # 32
NGROUPS = NC_ROWS // 128       # 128
NCHUNK = 6

_PROGRAM = None


def _build_program():
    import concourse.bacc as bacc
    import concourse.mybir as mybir
    import concourse.tile as tile

    f32 = mybir.dt.float32
    f32r = mybir.dt.float32r
    AF = mybir.ActivationFunctionType

    nc = bacc.Bacc(None, target_bir_lowering=False)
    X_d = nc.dram_tensor("X", [NC_ROWS, D], f32r, kind="ExternalInput")
    U_d = nc.dram_tensor("U", [128, NCHUNK, M], f32r, kind="ExternalInput")
    SEL_d = nc.dram_tensor("SEL", [128, 768], f32r, kind="ExternalInput")
    PAD_d = nc.dram_tensor("PAD", [96, TILE_ROWS], f32r, kind="ExternalInput")
    EYE_d = nc.dram_tensor("EYE", [128, 128], f32, kind="ExternalInput")
    EYER_d = nc.dram_tensor("EYER", [128, 128], f32r, kind="ExternalInput")
    OUT_d = nc.dram_tensor("out", [NC_ROWS], f32, kind="ExternalOutput")

    with tile.TileContext(nc) as tc:
        with (
            tc.tile_pool(name="const", bufs=1) as constp,
            tc.tile_pool(name="xin", bufs=3) as xinp,
            tc.tile_pool(name="xtp", bufs=2) as xtpool,
            tc.tile_pool(name="xt4p", bufs=2) as xt4pool,
            tc.tile_pool(name="xxp", bufs=2) as xxpool,
            tc.tile_pool(name="expp", bufs=4) as exppool,
            tc.tile_pool(name="sumsp", bufs=1) as sumspool,
            tc.tile_pool(name="finp", bufs=1) as finpool,
            tc.tile_pool(name="ps_xt", bufs=2, space="PSUM") as ps_xt,
            tc.tile_pool(name="ps_xt4", bufs=1, space="PSUM") as ps_xt4,
            tc.tile_pool(name="ps_rot", bufs=2, space="PSUM") as ps_rot,
            tc.tile_pool(name="ps_main", bufs=3, space="PSUM") as ps_main,
        ):
            U_sb = constp.tile([128, NCHUNK, M], f32r)
            nc.sync.dma_start(U_sb[:], U_d[:])
            SEL_sb = constp.tile([128, 768], f32r)
            nc.sync.dma_start(SEL_sb[:], SEL_d[:])
            EYE_sb = constp.tile([128, 128], f32)
            nc.sync.dma_start(EYE_sb[:], EYE_d[:])
            EYER_sb = constp.tile([128, 128], f32r)
            nc.sync.dma_start(EYER_sb[:], EYER_d[:])

            sums_sb = sumspool.tile([128, NGROUPS], f32)

            # persistent double-buffered X^T tiles: rows 32:128 hold the
            # constant [ones-row; zeros] pad, DMA'd once - per-tile writes
            # only touch rows 0:32, so the pad stays valid across reuse.
            xt_tiles = []
            for i in range(2):
                xt_p = xtpool.tile(
                    [128, TILE_ROWS], f32r, tag=f"xtP{i}", bufs=1, name=f"xt_p{i}"
                )
                nc.sync.dma_start(xt_p[32:128, :], PAD_d[:])
                xt_tiles.append(xt_p)

            for t in range(NTILES):
                x_t = xinp.tile([128, 4 * D], f32r, tag="x")
                nc.sync.dma_start(
                    x_t[:].rearrange("p (g d) -> p g d", g=4),
                    X_d[t * TILE_ROWS : (t + 1) * TILE_ROWS, :].rearrange(
                        "(g p) d -> p g d", p=128
                    ),
                )

                # X^T [32, 512] via 4 PE transposes
                xtps = ps_xt.tile([32, TILE_ROWS], f32r, tag="xtps")
                for g in range(4):
                    nc.tensor.transpose(
                        xtps[:, g * 128 : (g + 1) * 128],
                        x_t[:, g * D : (g + 1) * D],
                        EYER_sb[:],
                    )
                # xt_sb = [X^T (32) ; ones (1) ; zeros (95)] - serves both as
                # the sel-matmul moving operand (rows 32:128 exactly zero) and
                # as main-matmul chunk 5 (x-linear part + bias row).
                xt_sb = xt_tiles[t % 2]
                nc.scalar.copy(xt_sb[:32, :], xtps[:])

                # XT4 = 4-fold stack of X^T (partition p holds x_{p%32})
                xt4ps = ps_xt4.tile([128, TILE_ROWS], f32, tag="xt4ps")
                nc.tensor.matmul(
                    xt4ps[:], SEL_sb[:, 0:128], xt_sb[:], start=True, stop=True
                )
                xt4_sb = xt4pool.tile([128, TILE_ROWS], f32r, tag="xt4")
                nc.scalar.copy(xt4_sb[:], xt4ps[:])

                # pair-product chunks 0..3:
                #   chunk_c[p] = x_{p%32} * x_{(p%32 + 4c + p//32)%32}
                # chunk 4: k=16 block in rows 0:32, rows 32:128 exact zeros
                # (sel rows are zero there, and xt4 * 0 = 0).
                chunk_tiles = []
                for c in range(5):
                    rotps = ps_rot.tile([128, TILE_ROWS], f32, tag="rot")
                    nc.tensor.matmul(
                        rotps[:],
                        SEL_sb[:, 128 * (c + 1) : 128 * (c + 2)],
                        xt_sb[:],
                        start=True,
                        stop=True,
                    )
                    xx_c = xxpool.tile([128, TILE_ROWS], f32r, tag=f"xx{c}")
                    nc.vector.tensor_mul(xx_c[:], xt4_sb[:], rotps[:])
                    chunk_tiles.append(xx_c)
                chunk_tiles.append(xt_sb)  # chunk 5: [X^T; ones; zeros]

                # main accumulating matmuls (one open PSUM group per bank);
                # two 1-bank psum tiles (2 row-groups each) for deeper overlap
                for half in range(2):
                    psmain = ps_main.tile([128, 2 * M], f32, tag="main")
                    for s2 in range(2):
                        sub = half * 2 + s2
                        for c in range(NCHUNK):
                            nc.tensor.matmul(
                                psmain[:, s2 * M : (s2 + 1) * M],
                                chunk_tiles[c][:, sub * 128 : (sub + 1) * 128],
                                U_sb[:, c, :],
                                start=(c == 0),
                                stop=(c == NCHUNK - 1),
                            )
                    for s2 in range(2):
                        sub = half * 2 + s2
                        expsc = exppool.tile([128, M], f32, tag="exp")
                        col = t * 4 + sub
                        nc.scalar.activation(
                            expsc[:],
                            psmain[:, s2 * M : (s2 + 1) * M],
                            AF.Exp,
                            accum_out=sums_sb[:, col : col + 1],
                        )

            # epilogue: ll^T = Ln(sums); transpose; contiguous DMA out
            llT = finpool.tile([128, NGROUPS], f32)
            nc.scalar.activation(llT[:], sums_sb[:], AF.Ln)
            llps = ps_xt.tile([128, 128], f32, tag="xtps")
            nc.tensor.transpose(llps[:], llT[:], EYE_sb[:])
            ll_sb = finpool.tile([128, 128], f32)
            nc.scalar.copy(ll_sb[:], llps[:])
            nc.sync.dma_start(OUT_d.rearrange("(c p) -> c p", c=128), ll_sb[:])

    nc.compile()
    return nc


def _host_prep(center, cov_inv_sqrt, weight, threshold):
    L = np.asarray(cov_inv_sqrt, dtype=np.float64)
    w = np.abs(np.asarray(weight, dtype=np.float64))
    pr = w / w.sum()
    A = np.einsum("mij,mkj->mik", L, L)
    sign, logdet = np.linalg.slogdet(A)
    logcoef = np.log(pr) + 0.5 * logdet
    c64 = np.asarray(center, dtype=np.float64)
    Ac = np.einsum("mkl,ml->mk", A, c64)
    term3 = np.einsum("mk,mk->m", c64, Ac)
    bias = logcoef - 0.5 * term3 - float(np.asarray(threshold).reshape(-1)[0])

    U = np.zeros((128, NCHUNK, M), np.float32)
    p = np.arange(128)
    for c in range(4):
        k = 4 * c + p // 32
        d1 = p % 32
        d2 = (d1 + k) % 32
        mult = np.where((k == 0) | (k == 16), 1.0, 2.0)
        U[:, c, :] = (-0.5 * mult[:, None] * A[:, d1, d2].T).astype(np.float32)
    p32 = np.arange(32)
    U[:32, 4, :] = (-0.5 * A[:, p32, (p32 + 16) % 32].T).astype(np.float32)
    U[:32, 5, :] = Ac.T.astype(np.float32)
    U[32, 5, :] = bias.astype(np.float32)

    SEL = np.zeros((128, 768), np.float32)
    dd = np.arange(128)
    SEL[:, 0:128] = (dd[:, None] == (p[None, :] % 32)).astype(np.float32)
    for c in range(4):
        k = 4 * c + p // 32
        b = (p % 32 + k) % 32
        SEL[:, 128 * (c + 1) : 128 * (c + 2)] = (dd[:, None] == b[None, :]).astype(
            np.float32
        )
    b16 = np.where(p < 32, (p + 16) % 32, -1)
    SEL[:, 640:768] = (dd[:, None] == b16[None, :]).astype(np.float32)

    PAD = np.zeros((96, TILE_ROWS), np.float32)
    PAD[0, :] = 1.0
    EYE = np.eye(128, dtype=np.float32)
    return U, SEL, PAD, EYE


def kernel(X, center, cov_inv_sqrt, weight, threshold):
    global _PROGRAM
    from concourse.bass_utils import run_bass_kernel_spmd

    X = np.ascontiguousarray(np.asarray(X, dtype=np.float32))
    U, SEL, PAD, EYE = _host_prep(center, cov_inv_sqrt, weight, threshold)

    if _PROGRAM is None:
        _PROGRAM = _build_program()
    nc = _PROGRAM

    in_maps = []
    for k in range(NCORES):
        in_maps.append(
            {
                "X": X[k * NC_ROWS : (k + 1) * NC_ROWS],
                "U": U,
                "SEL": SEL,
                "PAD": PAD,
                "EYE": EYE,
                "EYER": EYE,
            }
        )
    res = run_bass_kernel_spmd(nc, in_maps, list(range(NCORES)))
    out = np.concatenate([res.results[k]["out"] for k in range(NCORES)])
    return out.astype(np.float32)
